# revision 15
# baseline (speedup 1.0000x reference)
"""Distributed Trainium2 kernel for nn_Encoder_88502096101469.

8-core SPMD layout (one NEFF, per-core data):
- Activations live TRANSPOSED in SBUF: X^T (512 feat x 512 cols), where
  cols 0-255 = batch-0 rows [256c, 256c+256) and cols 256-511 = batch-1
  rows [256c, 256c+256) for core c.
- Core c owns attention head h=c for BOTH batches. The torch-faithful
  "raw reshape" of (b, h, t, dv) -> (b, t, h*dv) maps head h's output to
  Z rows [256h, 256h+256) per batch, which is exactly core c's resident
  row range -> no post-attention exchange needed.
- Per batch, one 8-way AllToAll exchanges Q^T/K^T slices (64 head rows x
  local cols) and V natural slices (local rows x 64 head cols).
- All matmuls run in float32r (TF32-class, 4x faster than f32, ~1e-4 rel).
- Softmax skips max-subtraction (logits are O(1)); the denominator comes
  from a ones-column appended to V (lhsT M=65); exp folds the 1/8 scale.
- LayerNorm stats (feature axis = partitions) via ones-vector matmuls.
"""
import numpy as np

import concourse.bass as bass
import concourse.bacc as bacc
import concourse.tile as tile
from concourse import mybir
from concourse import bass_utils

NCORES = 8
DIM = 512
TLOC = 512          # per-core cols: 256 per batch
NITER = 3           # LAYERS + 1
LN_EPS = 1e-5

F32 = mybir.dt.float32
F32R = mybir.dt.float32r
AF = mybir.ActivationFunctionType
OP = mybir.AluOpType

# A2A per-batch shard layout (flat f32r words per (src,dst) pair):
#   [0:16384)      Q^T slice  (64 of-rows, 256 cols)
#   [16384:32768)  K^T slice  (64 of-rows, 256 cols)
#   [32768:49152)  V slice    (2 t-chunks, 128 rows, 64 fv-cols)
SHARD = 49152


def _build_graph(nc):
    xt_in = nc.dram_tensor("xt", [DIM, TLOC], F32R, kind="ExternalInput").ap()
    wq_in = nc.dram_tensor("wq", [DIM, DIM], F32R, kind="ExternalInput").ap()
    wk_in = nc.dram_tensor("wk", [DIM, DIM], F32R, kind="ExternalInput").ap()
    wv_in = nc.dram_tensor("wv", [DIM, DIM], F32R, kind="ExternalInput").ap()
    wo_in = nc.dram_tensor("wo", [DIM, DIM], F32R, kind="ExternalInput").ap()
    bq_in = nc.dram_tensor("bq", [128, 4], F32, kind="ExternalInput").ap()
    bk_in = nc.dram_tensor("bk", [128, 4], F32, kind="ExternalInput").ap()
    bo_in = nc.dram_tensor("bo", [128, 4], F32, kind="ExternalInput").ap()
    bv_in = nc.dram_tensor("bv", [1, DIM], F32R, kind="ExternalInput").ap()
    lng_in = nc.dram_tensor("lng", [128, 4], F32, kind="ExternalInput").ap()
    lnb_in = nc.dram_tensor("lnb", [128, 4], F32, kind="ExternalInput").ap()
    ones_in = nc.dram_tensor("ones", [128, 128], F32R, kind="ExternalInput").ap()
    ones3_in = nc.dram_tensor("ones3", [128, 16, 1], F32R, kind="ExternalInput").ap()
    out_d = nc.dram_tensor("out", [DIM, TLOC], F32R, kind="ExternalOutput").ap()

    groups = [list(range(NCORES))]

    from contextlib import ExitStack
    with tile.TileContext(nc) as tc, ExitStack() as ctx:
        const = ctx.enter_context(tc.tile_pool(name="const", bufs=1))
        act = ctx.enter_context(tc.tile_pool(name="act", bufs=1))
        qkv = ctx.enter_context(tc.tile_pool(name="qkv", bufs=1))
        gath = ctx.enter_context(tc.tile_pool(name="gath", bufs=1))
        epool = ctx.enter_context(tc.tile_pool(name="epool", bufs=3))
        small = ctx.enter_context(tc.tile_pool(name="small", bufs=1))
        dram = ctx.enter_context(tc.tile_pool(name="dram", bufs=1, space="DRAM"))
        s_psum = ctx.enter_context(tc.tile_pool(name="s_psum", bufs=2, space="PSUM"))
        o_psum = ctx.enter_context(tc.tile_pool(name="o_psum", bufs=2, space="PSUM"))
        ln_psum = ctx.enter_context(tc.tile_pool(name="ln_psum", bufs=2, space="PSUM"))
        if True:
            # ---- constants to SBUF ----
            def load_w(ap_in, nm):
                t = const.tile([128, 4, DIM], F32R, name=nm, tag=nm)
                nc.sync.dma_start(out=t, in_=ap_in.rearrange("(c p) f -> p c f", p=128))
                return t

            wq, wk, wv, wo = (load_w(wq_in, "wqt"), load_w(wk_in, "wkt"),
                              load_w(wv_in, "wvt"), load_w(wo_in, "wot"))
            bq = const.tile([128, 4], F32)
            bk = const.tile([128, 4], F32)
            bo = const.tile([128, 4], F32)
            lng = const.tile([128, 4], F32)
            lnb = const.tile([128, 4], F32)
            for t, a in ((bq, bq_in), (bk, bk_in), (bo, bo_in), (lng, lng_in), (lnb, lnb_in)):
                nc.sync.dma_start(out=t, in_=a)
            bv = const.tile([1, DIM], F32R)
            nc.sync.dma_start(out=bv, in_=bv_in)
            ones = const.tile([128, 128], F32R)
            nc.sync.dma_start(out=ones, in_=ones_in)
            ones3 = const.tile([128, 16, 1], F32R)
            nc.sync.dma_start(out=ones3, in_=ones3_in)
            eps_t = const.tile([1, 1], F32)
            nc.vector.memset(eps_t, LN_EPS)

            # initial activation
            x0 = act.tile([128, 4, TLOC], F32R, tag="resid")
            nc.sync.dma_start(out=x0, in_=xt_in.rearrange("(c p) f -> p c f", p=128))

            # DRAM bounce buffers for the per-batch A2A
            sendb = [dram.tile([NCORES, SHARD], F32R, tag=f"send{b}",
                               name=f"send{b}") for b in range(2)]
            recvb = [dram.tile([NCORES, SHARD], F32R, tag=f"recv{b}",
                               name=f"recv{b}") for b in range(2)]

            stat_d = dram.tile([2, DIM], F32, tag="stat")  # LN broadcast bounce

            def project_T(x, w, btile, tag):
                """(128,4,TLOC) f32r <- relu(w^T x + b), transposed output."""
                out = qkv.tile([128, 4, TLOC], F32R, tag=tag)
                for pair in range(2):
                    ps = s_psum.tile([128, 2, TLOC], F32, tag="s")
                    for i in range(2):
                        ofc = 2 * pair + i
                        for ifc in range(4):
                            nc.tensor.matmul(
                                ps[:, i, :],
                                w[:, ifc, 128 * ofc:128 * (ofc + 1)],
                                x[:, ifc, :],
                                start=(ifc == 0), stop=(ifc == 3))
                        nc.vector.tensor_scalar(
                            out=out[:, ofc, :], in0=ps[:, i, :],
                            scalar1=btile[:, ofc:ofc + 1], scalar2=0.0,
                            op0=OP.add, op1=OP.max)
                return out

            def project_V(x):
                """(128,4,DIM) f32r <- relu(x^T wv + bv), natural layout."""
                out = qkv.tile([128, 4, DIM], F32R, tag="v")
                for pair in range(2):
                    ps = s_psum.tile([128, 2, DIM], F32, tag="s")
                    for i in range(2):
                        tch = 2 * pair + i
                        for ifc in range(4):
                            nc.tensor.matmul(
                                ps[:, i, :],
                                x[:, ifc, 128 * tch:128 * (tch + 1)],
                                wv[:, ifc, :],
                                start=(ifc == 0), stop=False)
                        nc.tensor.matmul(
                            ps[:, i, :], ones[0:1, :], bv,
                            start=False, stop=True)
                        nc.vector.tensor_scalar(
                            out=out[:, tch, :], in0=ps[:, i, :],
                            scalar1=0.0, scalar2=None, op0=OP.max)
                return out

            def mha(x):
                qt = project_T(x, wq, bq, "qt")
                kt = project_T(x, wk, bk, "kt")
                v = project_V(x)

                # pack + exchange, per batch
                qth, kth, v1 = [], [], []
                for b in range(2):
                    sb, rb = sendb[b], recvb[b]
                    for d in range(NCORES):
                        cq, p0 = d // 2, 64 * (d % 2)
                        nc.sync.dma_start(
                            out=sb[d, 0:16384].rearrange("(r c) -> r c", c=256),
                            in_=qt[p0:p0 + 64, cq, 256 * b:256 * (b + 1)])
                        nc.sync.dma_start(
                            out=sb[d, 16384:32768].rearrange("(r c) -> r c", c=256),
                            in_=kt[p0:p0 + 64, cq, 256 * b:256 * (b + 1)])
                        nc.sync.dma_start(
                            out=sb[d, 32768:49152].rearrange(
                                "(tc p j) -> p tc j", tc=2, p=128),
                            in_=v[:, 2 * b:2 * b + 2, 64 * d:64 * (d + 1)])
                    nc.gpsimd.collective_compute(
                        "AllToAll", OP.bypass, replica_groups=groups,
                        ins=[sb.opt()], outs=[rb.opt()])
                    qh = gath.tile([64, NCORES, 256], F32R, tag=f"qh{b}")
                    kh = gath.tile([64, NCORES, 256], F32R, tag=f"kh{b}")
                    vh = gath.tile([128, 16, 65], F32R, tag=f"vh{b}")
                    nc.sync.dma_start(
                        out=qh, in_=rb[:, 0:16384].rearrange("s (r c) -> r s c", r=64))
                    nc.sync.dma_start(
                        out=kh, in_=rb[:, 16384:32768].rearrange("s (r c) -> r s c", r=64))
                    for tc in range(2):
                        nc.sync.dma_start(
                            out=vh[:, tc::2, 0:64],
                            in_=rb[:, 32768 + 8192 * tc:32768 + 8192 * (tc + 1)]
                                .rearrange("s (p j) -> p s j", p=128))
                    nc.sync.dma_start(out=vh[:, :, 64:65], in_=ones3)
                    qth.append(qh); kth.append(kh); v1.append(vh)

                # attention; Z^T built in-place
                z = qkv.tile([128, 4, TLOC], F32R, tag="z")
                for b in range(2):
                    for j in range(4):
                        ops = o_psum.tile([65, 512], F32, tag="o")
                        for g in range(8):
                            sps = s_psum.tile([128, 2, 512], F32, tag="s")
                            for u in range(2):
                                k = 2 * g + u
                                nc.tensor.matmul(
                                    sps[:, u, :],
                                    kth[b][:, k // 2, 128 * (k % 2):128 * (k % 2 + 1)],
                                    qth[b][:, 2 * j:2 * j + 2, :],
                                    start=True, stop=True)
                            e = epool.tile([128, 2, 512], F32R, tag="e")
                            nc.scalar.activation(e, sps, AF.Exp, scale=0.125)
                            for u in range(2):
                                k = 2 * g + u
                                nc.tensor.matmul(
                                    ops, v1[b][:, k, :], e[:, u, :],
                                    start=(k == 0), stop=(k == 15))
                        recip = small.tile([1, 512], F32, tag="recip", bufs=2)
                        nc.vector.reciprocal(recip, ops[64:65, :])
                        # broadcast recip over 64 partitions via DRAM bounce
                        nc.sync.dma_start(out=stat_d[0:1, :], in_=recip)
                        rrep = small.tile([64, 512], F32, tag="rrep", bufs=2)
                        nc.sync.dma_start(
                            out=rrep,
                            in_=stat_d[0:1, :].partition_broadcast(64)[:, 0, :])
                        for s8 in range(8):
                            nc.vector.tensor_tensor(
                                out=z[64 * (s8 % 2):64 * (s8 % 2) + 64, s8 // 2,
                                      256 * b + 64 * j:256 * b + 64 * (j + 1)],
                                in0=ops[0:64, s8::8],
                                in1=rrep[:, s8::8],
                                op=OP.mult)
                return project_T(z, wo, bo, "y")

            def layer_norm(x, resid=None):
                """LN over features (partitions); optional residual add first."""
                if resid is not None:
                    xr = act.tile([128, 4, TLOC], F32R, tag="xr")
                    nc.vector.tensor_tensor(out=xr, in0=x, in1=resid, op=OP.add)
                    x = xr
                x2 = act.tile([128, 4, TLOC], F32R, tag="x2")
                nc.vector.tensor_tensor(out=x2, in0=x, in1=x, op=OP.mult)
                mu_ps = ln_psum.tile([1, TLOC], F32, tag="ln")
                for ifc in range(4):
                    nc.tensor.matmul(mu_ps, ones[:, 0:1], x[:, ifc, :],
                                     start=(ifc == 0), stop=(ifc == 3))
                m2_ps = ln_psum.tile([1, TLOC], F32, tag="ln")
                for ifc in range(4):
                    nc.tensor.matmul(m2_ps, ones[:, 0:1], x2[:, ifc, :],
                                     start=(ifc == 0), stop=(ifc == 3))
                mu = small.tile([1, TLOC], F32, tag="mu", bufs=2)
                ex2 = small.tile([1, TLOC], F32, tag="ex2", bufs=2)
                nc.vector.tensor_scalar(out=mu, in0=mu_ps, scalar1=1.0 / DIM,
                                        scalar2=None, op0=OP.mult)
                nc.vector.tensor_scalar(out=ex2, in0=m2_ps, scalar1=1.0 / DIM,
                                        scalar2=None, op0=OP.mult)
                m2 = small.tile([1, TLOC], F32, tag="m2", bufs=2)
                nc.vector.tensor_tensor(out=m2, in0=mu, in1=mu, op=OP.mult)
                nc.vector.tensor_tensor(out=ex2, in0=ex2, in1=m2, op=OP.subtract)
                sd = small.tile([1, TLOC], F32, tag="sd", bufs=2)
                nc.scalar.activation(sd, ex2, AF.Sqrt, bias=eps_t)
                nc.vector.reciprocal(sd, sd)          # sd now holds rstd
                nc.vector.tensor_tensor(out=mu, in0=mu, in1=sd, op=OP.mult)
                # broadcast rstd & mu*rstd across partitions via DRAM bounce
                nc.sync.dma_start(out=stat_d[0:1, :], in_=sd)
                nc.sync.dma_start(out=stat_d[1:2, :], in_=mu)
                srep = small.tile([128, 2, TLOC], F32, tag="srep", bufs=2)
                nc.sync.dma_start(
                    out=srep, in_=stat_d.partition_broadcast(128))
                out = act.tile([128, 4, TLOC], F32R, tag="lnout", bufs=2)
                for ifc in range(4):
                    t1 = small.tile([128, TLOC], F32, tag="t1", bufs=2)
                    nc.vector.tensor_tensor(out=t1, in0=x[:, ifc, :],
                                            in1=srep[:, 0, :], op=OP.mult)
                    nc.vector.tensor_tensor(out=t1, in0=t1, in1=srep[:, 1, :],
                                            op=OP.subtract)
                    nc.vector.tensor_scalar(
                        out=out[:, ifc, :], in0=t1,
                        scalar1=lng[:, ifc:ifc + 1], scalar2=lnb[:, ifc:ifc + 1],
                        op0=OP.mult, op1=OP.add)
                return out

            init = x0
            for _ in range(NITER):
                o1 = mha(init)
                l1 = layer_norm(o1)
                o2 = mha(l1)
                init = layer_norm(o2, resid=init)

            nc.sync.dma_start(
                out=out_d.rearrange("(c p) f -> p c f", p=128), in_=init)
    return nc


_NC_CACHE = None


def _get_nc():
    global _NC_CACHE
    if _NC_CACHE is None:
        nc = bacc.Bacc("TRN2", target_bir_lowering=False, debug=False,
                       num_devices=NCORES)
        _build_graph(nc)
        nc.compile()
        _NC_CACHE = nc
    return _NC_CACHE


def kernel(encoder_inputs, Wq, bq, Wk, bk, Wv, bv, Wo, bo, ln_g, ln_b,
           _trace=False, _trace_kwargs=None):
    x = np.asarray(encoder_inputs, dtype=np.float32)
    consts = {
        "wq": np.ascontiguousarray(np.asarray(Wq, np.float32)),
        "wk": np.ascontiguousarray(np.asarray(Wk, np.float32)),
        "wv": np.ascontiguousarray(np.asarray(Wv, np.float32)),
        "wo": np.ascontiguousarray(np.asarray(Wo, np.float32)),
        "bq": np.ascontiguousarray(np.asarray(bq, np.float32).reshape(4, 128).T),
        "bk": np.ascontiguousarray(np.asarray(bk, np.float32).reshape(4, 128).T),
        "bo": np.ascontiguousarray(np.asarray(bo, np.float32).reshape(4, 128).T),
        "bv": np.asarray(bv, np.float32).reshape(1, DIM),
        "lng": np.ascontiguousarray(np.asarray(ln_g, np.float32).reshape(4, 128).T),
        "lnb": np.ascontiguousarray(np.asarray(ln_b, np.float32).reshape(4, 128).T),
        "ones": np.ones((128, 128), np.float32),
        "ones3": np.ones((128, 16, 1), np.float32),
    }
    in_maps = []
    for c in range(NCORES):
        xt = np.concatenate([x[0, 256 * c:256 * (c + 1)].T,
                             x[1, 256 * c:256 * (c + 1)].T], axis=1)
        in_maps.append({"xt": np.ascontiguousarray(xt), **consts})

    nc = _get_nc()
    res = bass_utils.run_bass_kernel_spmd(
        nc, in_maps, core_ids=list(range(NCORES)),
        trace=_trace, **(_trace_kwargs or {}))

    out = np.zeros((2, 2048, DIM), np.float32)
    for c in range(NCORES):
        r = res.results[c]["out"]
        out[0, 256 * c:256 * (c + 1)] = r[:, :256].T
        out[1, 256 * c:256 * (c + 1)] = r[:, 256:].T
    if _trace:
        kernel._last_results = res
    return out


# revision 20
# speedup vs baseline: 1.2368x; 1.2368x over previous
"""Distributed Trainium2 kernel for nn_Encoder_88502096101469.

8-core SPMD layout (one NEFF, per-core data):
- Activations live TRANSPOSED in SBUF: X^T (512 feat x 512 cols), where
  cols 0-255 = batch-0 rows [256c, 256c+256) and cols 256-511 = batch-1
  rows [256c, 256c+256) for core c.
- Core c owns attention head h=c for BOTH batches. The torch-faithful
  "raw reshape" of (b, h, t, dv) -> (b, t, h*dv) maps head h's output to
  Z rows [256h, 256h+256) per batch, which is exactly core c's resident
  row range -> no post-attention exchange needed.
- Per batch, one 8-way AllToAll exchanges Q^T/K^T slices (64 head rows x
  local cols) and V natural slices (local rows x 64 head cols).
- All matmuls run in float32r (TF32-class, 4x faster than f32, ~1e-4 rel).
- Softmax skips max-subtraction (logits are O(1)); the denominator comes
  from a ones-column appended to V (lhsT M=65); exp folds the 1/8 scale.
- LayerNorm stats (feature axis = partitions) via ones-vector matmuls.
"""
import numpy as np
import ml_dtypes

import concourse.bass as bass
import concourse.bacc as bacc
import concourse.tile as tile
from concourse import mybir
from concourse import bass_utils

NCORES = 8
DIM = 512
TLOC = 512          # per-core cols: 256 per batch
NITER = 3           # LAYERS + 1
LN_EPS = 1e-5

F32 = mybir.dt.float32
F32R = mybir.dt.float32r
BF16 = mybir.dt.bfloat16
AF = mybir.ActivationFunctionType
OP = mybir.AluOpType

# A2A per-batch shard layout (flat f32r words per (src,dst) pair):
#   [0:16384)      Q^T slice  (64 of-rows, 256 cols)
#   [16384:32768)  K^T slice  (64 of-rows, 256 cols)
#   [32768:49152)  V slice    (2 t-chunks, 128 rows, 64 fv-cols)
SHARD = 49152


def _build_graph(nc):
    xt_in = nc.dram_tensor("xt", [DIM, TLOC], F32R, kind="ExternalInput").ap()
    wq_in = nc.dram_tensor("wq", [DIM, DIM], F32R, kind="ExternalInput").ap()
    wk_in = nc.dram_tensor("wk", [DIM, DIM], F32R, kind="ExternalInput").ap()
    wv_in = nc.dram_tensor("wv", [DIM, DIM], F32R, kind="ExternalInput").ap()
    wo_in = nc.dram_tensor("wo", [DIM, DIM], F32R, kind="ExternalInput").ap()
    bq_in = nc.dram_tensor("bq", [128, 4], F32, kind="ExternalInput").ap()
    bk_in = nc.dram_tensor("bk", [128, 4], F32, kind="ExternalInput").ap()
    bo_in = nc.dram_tensor("bo", [128, 4], F32, kind="ExternalInput").ap()
    bv_in = nc.dram_tensor("bv", [1, DIM], F32R, kind="ExternalInput").ap()
    lng_in = nc.dram_tensor("lng", [128, 4], F32, kind="ExternalInput").ap()
    lnb_in = nc.dram_tensor("lnb", [128, 4], F32, kind="ExternalInput").ap()
    ones_in = nc.dram_tensor("ones", [128, 128], F32R, kind="ExternalInput").ap()
    ones3_in = nc.dram_tensor("ones3", [128, 16, 1], BF16, kind="ExternalInput").ap()
    out_d = nc.dram_tensor("out", [DIM, TLOC], F32R, kind="ExternalOutput").ap()

    groups = [list(range(NCORES))]

    from contextlib import ExitStack
    with tile.TileContext(nc) as tc, ExitStack() as ctx:
        const = ctx.enter_context(tc.tile_pool(name="const", bufs=1))
        act = ctx.enter_context(tc.tile_pool(name="act", bufs=1))
        qkv = ctx.enter_context(tc.tile_pool(name="qkv", bufs=1))
        gath = ctx.enter_context(tc.tile_pool(name="gath", bufs=1))
        epool = ctx.enter_context(tc.tile_pool(name="epool", bufs=3))
        small = ctx.enter_context(tc.tile_pool(name="small", bufs=1))
        dram = ctx.enter_context(tc.tile_pool(name="dram", bufs=1, space="DRAM"))
        s_psum = ctx.enter_context(tc.tile_pool(name="s_psum", bufs=2, space="PSUM"))
        o_psum = ctx.enter_context(tc.tile_pool(name="o_psum", bufs=2, space="PSUM"))
        ln_psum = ctx.enter_context(tc.tile_pool(name="ln_psum", bufs=2, space="PSUM"))
        if True:
            # ---- constants to SBUF ----
            def load_w(ap_in, nm):
                t = const.tile([128, 4, DIM], F32R, name=nm, tag=nm)
                nc.sync.dma_start(out=t, in_=ap_in.rearrange("(c p) f -> p c f", p=128))
                return t

            wq, wk, wv, wo = (load_w(wq_in, "wqt"), load_w(wk_in, "wkt"),
                              load_w(wv_in, "wvt"), load_w(wo_in, "wot"))
            bq = const.tile([128, 4], F32)
            bk = const.tile([128, 4], F32)
            bo = const.tile([128, 4], F32)
            lng = const.tile([128, 4], F32)
            lnb = const.tile([128, 4], F32)
            for t, a in ((bq, bq_in), (bk, bk_in), (bo, bo_in), (lng, lng_in), (lnb, lnb_in)):
                nc.sync.dma_start(out=t, in_=a)
            bv = const.tile([1, DIM], F32R)
            nc.sync.dma_start(out=bv, in_=bv_in)
            ones = const.tile([128, 128], F32R)
            nc.sync.dma_start(out=ones, in_=ones_in)
            ones3 = const.tile([128, 16, 1], BF16)
            nc.sync.dma_start(out=ones3, in_=ones3_in)
            eps_t = const.tile([1, 1], F32)
            nc.vector.memset(eps_t, LN_EPS)

            # initial activation
            x0 = act.tile([128, 4, TLOC], F32R, tag="resid")
            nc.sync.dma_start(out=x0, in_=xt_in.rearrange("(c p) f -> p c f", p=128))

            # DRAM bounce buffers for the per-batch A2A
            sendb = [dram.tile([NCORES, SHARD], BF16, tag=f"send{b}",
                               name=f"send{b}") for b in range(2)]
            recvb = [dram.tile([NCORES, SHARD], BF16, tag=f"recv{b}",
                               name=f"recv{b}") for b in range(2)]

            stat_d = dram.tile([2, DIM], F32, tag="stat")  # LN broadcast bounce

            def project_T(x, w, btile, tag, odt=F32R):
                """(128,4,TLOC) <- relu(w^T x + b), transposed output."""
                out = qkv.tile([128, 4, TLOC], odt, tag=tag)
                for pair in range(2):
                    ps = s_psum.tile([128, 2, TLOC], F32, tag="s")
                    for i in range(2):
                        ofc = 2 * pair + i
                        for ifc in range(4):
                            nc.tensor.matmul(
                                ps[:, i, :],
                                w[:, ifc, 128 * ofc:128 * (ofc + 1)],
                                x[:, ifc, :],
                                start=(ifc == 0), stop=(ifc == 3))
                        nc.vector.tensor_scalar(
                            out=out[:, ofc, :], in0=ps[:, i, :],
                            scalar1=btile[:, ofc:ofc + 1], scalar2=0.0,
                            op0=OP.add, op1=OP.max)
                return out

            def project_V(x):
                """(128,4,DIM) f32r <- relu(x^T wv + bv), natural layout."""
                out = qkv.tile([128, 4, DIM], BF16, tag="v")
                for pair in range(2):
                    ps = s_psum.tile([128, 2, DIM], F32, tag="s")
                    for i in range(2):
                        tch = 2 * pair + i
                        for ifc in range(4):
                            nc.tensor.matmul(
                                ps[:, i, :],
                                x[:, ifc, 128 * tch:128 * (tch + 1)],
                                wv[:, ifc, :],
                                start=(ifc == 0), stop=False)
                        nc.tensor.matmul(
                            ps[:, i, :], ones[0:1, :], bv,
                            start=False, stop=True)
                        nc.vector.tensor_scalar(
                            out=out[:, tch, :], in0=ps[:, i, :],
                            scalar1=0.0, scalar2=None, op0=OP.max)
                return out

            def mha(x):
                qt = project_T(x, wq, bq, "qt", odt=BF16)
                kt = project_T(x, wk, bk, "kt", odt=BF16)
                v = project_V(x)

                # pack + exchange, per batch
                qth, kth, v1 = [], [], []
                for b in range(2):
                    sb, rb = sendb[b], recvb[b]
                    for d in range(NCORES):
                        cq, p0 = d // 2, 64 * (d % 2)
                        nc.sync.dma_start(
                            out=sb[d, 0:16384].rearrange("(r c) -> r c", c=256),
                            in_=qt[p0:p0 + 64, cq, 256 * b:256 * (b + 1)])
                        nc.sync.dma_start(
                            out=sb[d, 16384:32768].rearrange("(r c) -> r c", c=256),
                            in_=kt[p0:p0 + 64, cq, 256 * b:256 * (b + 1)])
                        nc.sync.dma_start(
                            out=sb[d, 32768:49152].rearrange(
                                "(tc p j) -> p tc j", tc=2, p=128),
                            in_=v[:, 2 * b:2 * b + 2, 64 * d:64 * (d + 1)])
                    nc.gpsimd.collective_compute(
                        "AllToAll", OP.bypass, replica_groups=groups,
                        ins=[sb.opt()], outs=[rb.opt()])
                    qh = gath.tile([64, NCORES, 256], BF16, tag=f"qh{b}")
                    kh = gath.tile([64, NCORES, 256], BF16, tag=f"kh{b}")
                    vh = gath.tile([128, 16, 65], BF16, tag=f"vh{b}")
                    nc.sync.dma_start(
                        out=qh, in_=rb[:, 0:16384].rearrange("s (r c) -> r s c", r=64))
                    nc.sync.dma_start(
                        out=kh, in_=rb[:, 16384:32768].rearrange("s (r c) -> r s c", r=64))
                    for tc in range(2):
                        nc.sync.dma_start(
                            out=vh[:, tc::2, 0:64],
                            in_=rb[:, 32768 + 8192 * tc:32768 + 8192 * (tc + 1)]
                                .rearrange("s (p j) -> p s j", p=128))
                    nc.sync.dma_start(out=vh[:, :, 64:65], in_=ones3)
                    qth.append(qh); kth.append(kh); v1.append(vh)

                # attention; Z^T built in-place
                z = qkv.tile([128, 4, TLOC], F32R, tag="z")
                for b in range(2):
                    for j in range(4):
                        ops = o_psum.tile([65, 512], F32, tag="o")
                        for g in range(8):
                            sps = s_psum.tile([128, 2, 512], F32, tag="s")
                            for u in range(2):
                                k = 2 * g + u
                                nc.tensor.matmul(
                                    sps[:, u, :],
                                    kth[b][:, k // 2, 128 * (k % 2):128 * (k % 2 + 1)],
                                    qth[b][:, 2 * j:2 * j + 2, :],
                                    start=True, stop=True)
                            e = epool.tile([128, 2, 512], BF16, tag="e")
                            nc.scalar.activation(e, sps, AF.Exp, scale=0.125)
                            for u in range(2):
                                k = 2 * g + u
                                nc.tensor.matmul(
                                    ops, v1[b][:, k, :], e[:, u, :],
                                    start=(k == 0), stop=(k == 15))
                        rsum = small.tile([1, 512], F32, tag="rsum", bufs=2)
                        nc.vector.tensor_copy(rsum, ops[64:65, :])
                        recip = small.tile([1, 512], F32, tag="recip", bufs=2)
                        nc.vector.reciprocal_approx_fast(recip, rsum)
                        # broadcast recip over 64 partitions via DRAM bounce
                        nc.sync.dma_start(out=stat_d[0:1, :], in_=recip)
                        rrep = small.tile([64, 512], F32, tag="rrep", bufs=2)
                        nc.sync.dma_start(
                            out=rrep,
                            in_=stat_d[0:1, :].partition_broadcast(64)[:, 0, :])
                        o_v = ops[0:64, :].rearrange("f (r s) -> f s r", s=8)
                        r_v = rrep.rearrange("f (r s) -> f s r", s=8)
                        for q in range(2):
                            nc.vector.tensor_tensor(
                                out=z[64 * q:64 * (q + 1), :,
                                      256 * b + 64 * j:256 * b + 64 * (j + 1)],
                                in0=o_v[:, q::2, :],
                                in1=r_v[:, q::2, :],
                                op=OP.mult)
                return project_T(z, wo, bo, "y")

            def layer_norm(x, resid=None):
                """LN over features (partitions); optional residual add first."""
                if resid is not None:
                    xr = act.tile([128, 4, TLOC], F32R, tag="xr")
                    nc.vector.tensor_tensor(out=xr, in0=x, in1=resid, op=OP.add)
                    x = xr
                x2 = act.tile([128, 4, TLOC], F32R, tag="x2")
                nc.vector.tensor_tensor(out=x2, in0=x, in1=x, op=OP.mult)
                mu_ps = ln_psum.tile([1, TLOC], F32, tag="ln")
                for ifc in range(4):
                    nc.tensor.matmul(mu_ps, ones[:, 0:1], x[:, ifc, :],
                                     start=(ifc == 0), stop=(ifc == 3))
                m2_ps = ln_psum.tile([1, TLOC], F32, tag="ln")
                for ifc in range(4):
                    nc.tensor.matmul(m2_ps, ones[:, 0:1], x2[:, ifc, :],
                                     start=(ifc == 0), stop=(ifc == 3))
                mu = small.tile([1, TLOC], F32, tag="mu", bufs=2)
                ex2 = small.tile([1, TLOC], F32, tag="ex2", bufs=2)
                nc.vector.tensor_scalar(out=mu, in0=mu_ps, scalar1=1.0 / DIM,
                                        scalar2=None, op0=OP.mult)
                nc.vector.tensor_scalar(out=ex2, in0=m2_ps, scalar1=1.0 / DIM,
                                        scalar2=None, op0=OP.mult)
                m2 = small.tile([1, TLOC], F32, tag="m2", bufs=2)
                nc.vector.tensor_tensor(out=m2, in0=mu, in1=mu, op=OP.mult)
                nc.vector.tensor_tensor(out=ex2, in0=ex2, in1=m2, op=OP.subtract)
                sd = small.tile([1, TLOC], F32, tag="sd", bufs=2)
                nc.scalar.activation(sd, ex2, AF.Sqrt, bias=eps_t)
                nc.vector.reciprocal(sd, sd)          # sd now holds rstd
                nc.vector.tensor_tensor(out=mu, in0=mu, in1=sd, op=OP.mult)
                # broadcast rstd & mu*rstd across partitions via DRAM bounce
                nc.sync.dma_start(out=stat_d[0:1, :], in_=sd)
                nc.sync.dma_start(out=stat_d[1:2, :], in_=mu)
                srep = small.tile([128, 2, TLOC], F32, tag="srep", bufs=2)
                nc.sync.dma_start(
                    out=srep, in_=stat_d.partition_broadcast(128))
                out = act.tile([128, 4, TLOC], F32R, tag="lnout", bufs=2)
                for ifc in range(4):
                    t1 = small.tile([128, TLOC], F32, tag="t1", bufs=2)
                    nc.vector.tensor_tensor(out=t1, in0=x[:, ifc, :],
                                            in1=srep[:, 0, :], op=OP.mult)
                    nc.vector.tensor_tensor(out=t1, in0=t1, in1=srep[:, 1, :],
                                            op=OP.subtract)
                    nc.vector.tensor_scalar(
                        out=out[:, ifc, :], in0=t1,
                        scalar1=lng[:, ifc:ifc + 1], scalar2=lnb[:, ifc:ifc + 1],
                        op0=OP.mult, op1=OP.add)
                return out

            init = x0
            for _ in range(NITER):
                o1 = mha(init)
                l1 = layer_norm(o1)
                o2 = mha(l1)
                init = layer_norm(o2, resid=init)

            nc.sync.dma_start(
                out=out_d.rearrange("(c p) f -> p c f", p=128), in_=init)
    return nc


_NC_CACHE = None


def _get_nc():
    global _NC_CACHE
    if _NC_CACHE is None:
        nc = bacc.Bacc("TRN2", target_bir_lowering=False, debug=False,
                       num_devices=NCORES)
        _build_graph(nc)
        nc.compile()
        _NC_CACHE = nc
    return _NC_CACHE


def kernel(encoder_inputs, Wq, bq, Wk, bk, Wv, bv, Wo, bo, ln_g, ln_b,
           _trace=False, _trace_kwargs=None):
    x = np.asarray(encoder_inputs, dtype=np.float32)
    consts = {
        "wq": np.ascontiguousarray(np.asarray(Wq, np.float32)),
        "wk": np.ascontiguousarray(np.asarray(Wk, np.float32)),
        "wv": np.ascontiguousarray(np.asarray(Wv, np.float32)),
        "wo": np.ascontiguousarray(np.asarray(Wo, np.float32)),
        "bq": np.ascontiguousarray(np.asarray(bq, np.float32).reshape(4, 128).T),
        "bk": np.ascontiguousarray(np.asarray(bk, np.float32).reshape(4, 128).T),
        "bo": np.ascontiguousarray(np.asarray(bo, np.float32).reshape(4, 128).T),
        "bv": np.asarray(bv, np.float32).reshape(1, DIM),
        "lng": np.ascontiguousarray(np.asarray(ln_g, np.float32).reshape(4, 128).T),
        "lnb": np.ascontiguousarray(np.asarray(ln_b, np.float32).reshape(4, 128).T),
        "ones": np.ones((128, 128), np.float32),
        "ones3": np.ones((128, 16, 1), ml_dtypes.bfloat16),
    }
    in_maps = []
    for c in range(NCORES):
        xt = np.concatenate([x[0, 256 * c:256 * (c + 1)].T,
                             x[1, 256 * c:256 * (c + 1)].T], axis=1)
        in_maps.append({"xt": np.ascontiguousarray(xt), **consts})

    nc = _get_nc()
    res = bass_utils.run_bass_kernel_spmd(
        nc, in_maps, core_ids=list(range(NCORES)),
        trace=_trace, **(_trace_kwargs or {}))

    out = np.zeros((2, 2048, DIM), np.float32)
    for c in range(NCORES):
        r = res.results[c]["out"]
        out[0, 256 * c:256 * (c + 1)] = r[:, :256].T
        out[1, 256 * c:256 * (c + 1)] = r[:, 256:].T
    if _trace:
        kernel._last_results = res
    return out


# revision 22
# speedup vs baseline: 1.2490x; 1.0098x over previous
"""Distributed Trainium2 kernel for nn_Encoder_88502096101469.

8-core SPMD layout (one NEFF, per-core data):
- Activations live TRANSPOSED in SBUF: X^T (512 feat x 512 cols), where
  cols 0-255 = batch-0 rows [256c, 256c+256) and cols 256-511 = batch-1
  rows [256c, 256c+256) for core c.
- Core c owns attention head h=c for BOTH batches. The torch-faithful
  "raw reshape" of (b, h, t, dv) -> (b, t, h*dv) maps head h's output to
  Z rows [256h, 256h+256) per batch, which is exactly core c's resident
  row range -> no post-attention exchange needed.
- Per batch, one 8-way AllToAll exchanges Q^T/K^T slices (64 head rows x
  local cols) and V natural slices (local rows x 64 head cols).
- All matmuls run in float32r (TF32-class, 4x faster than f32, ~1e-4 rel).
- Softmax skips max-subtraction (logits are O(1)); the denominator comes
  from a ones-column appended to V (lhsT M=65); exp folds the 1/8 scale.
- LayerNorm stats (feature axis = partitions) via ones-vector matmuls.
"""
import numpy as np
import ml_dtypes

import concourse.bass as bass
import concourse.bacc as bacc
import concourse.tile as tile
from concourse import mybir
from concourse import bass_utils

NCORES = 8
DIM = 512
TLOC = 512          # per-core cols: 256 per batch
NITER = 3           # LAYERS + 1
LN_EPS = 1e-5

F32 = mybir.dt.float32
F32R = mybir.dt.float32r
BF16 = mybir.dt.bfloat16
AF = mybir.ActivationFunctionType
OP = mybir.AluOpType

# A2A per-batch shard layout (flat f32r words per (src,dst) pair):
#   [0:16384)      Q^T slice  (64 of-rows, 256 cols)
#   [16384:32768)  K^T slice  (64 of-rows, 256 cols)
#   [32768:49152)  V slice    (2 t-chunks, 128 rows, 64 fv-cols)
SHARD = 49152


def _build_graph(nc):
    xt_in = nc.dram_tensor("xt", [DIM, TLOC], F32R, kind="ExternalInput").ap()
    wq_in = nc.dram_tensor("wq", [DIM, DIM], F32R, kind="ExternalInput").ap()
    wk_in = nc.dram_tensor("wk", [DIM, DIM], F32R, kind="ExternalInput").ap()
    wv_in = nc.dram_tensor("wv", [DIM, DIM], F32R, kind="ExternalInput").ap()
    wo_in = nc.dram_tensor("wo", [DIM, DIM], F32R, kind="ExternalInput").ap()
    bq_in = nc.dram_tensor("bq", [128, 4], F32, kind="ExternalInput").ap()
    bk_in = nc.dram_tensor("bk", [128, 4], F32, kind="ExternalInput").ap()
    bo_in = nc.dram_tensor("bo", [128, 4], F32, kind="ExternalInput").ap()
    bv_in = nc.dram_tensor("bv", [1, DIM], F32R, kind="ExternalInput").ap()
    lng_in = nc.dram_tensor("lng", [128, 4], F32, kind="ExternalInput").ap()
    lnb_in = nc.dram_tensor("lnb", [128, 4], F32, kind="ExternalInput").ap()
    ones_in = nc.dram_tensor("ones", [128, 128], F32R, kind="ExternalInput").ap()
    ones3_in = nc.dram_tensor("ones3", [128, 16, 1], BF16, kind="ExternalInput").ap()
    out_d = nc.dram_tensor("out", [DIM, TLOC], F32R, kind="ExternalOutput").ap()

    groups = [list(range(NCORES))]

    from contextlib import ExitStack
    with tile.TileContext(nc) as tc, ExitStack() as ctx:
        const = ctx.enter_context(tc.tile_pool(name="const", bufs=1))
        act = ctx.enter_context(tc.tile_pool(name="act", bufs=1))
        qkv = ctx.enter_context(tc.tile_pool(name="qkv", bufs=1))
        gath = ctx.enter_context(tc.tile_pool(name="gath", bufs=1))
        epool = ctx.enter_context(tc.tile_pool(name="epool", bufs=3))
        small = ctx.enter_context(tc.tile_pool(name="small", bufs=1))
        dram = ctx.enter_context(tc.tile_pool(name="dram", bufs=1, space="DRAM"))
        s_psum = ctx.enter_context(tc.tile_pool(name="s_psum", bufs=2, space="PSUM"))
        o_psum = ctx.enter_context(tc.tile_pool(name="o_psum", bufs=2, space="PSUM"))
        ln_psum = ctx.enter_context(tc.tile_pool(name="ln_psum", bufs=2, space="PSUM"))
        if True:
            # ---- constants to SBUF ----
            def load_w(ap_in, nm):
                t = const.tile([128, 4, DIM], F32R, name=nm, tag=nm)
                nc.sync.dma_start(out=t, in_=ap_in.rearrange("(c p) f -> p c f", p=128))
                return t

            wq, wk, wv, wo = (load_w(wq_in, "wqt"), load_w(wk_in, "wkt"),
                              load_w(wv_in, "wvt"), load_w(wo_in, "wot"))
            bq = const.tile([128, 4], F32)
            bk = const.tile([128, 4], F32)
            bo = const.tile([128, 4], F32)
            lng = const.tile([128, 4], F32)
            lnb = const.tile([128, 4], F32)
            for t, a in ((bq, bq_in), (bk, bk_in), (bo, bo_in), (lng, lng_in), (lnb, lnb_in)):
                nc.sync.dma_start(out=t, in_=a)
            bv = const.tile([1, DIM], F32R)
            nc.sync.dma_start(out=bv, in_=bv_in)
            ones = const.tile([128, 128], F32R)
            nc.sync.dma_start(out=ones, in_=ones_in)
            ones3 = const.tile([128, 16, 1], BF16)
            nc.sync.dma_start(out=ones3, in_=ones3_in)
            eps_t = const.tile([1, 1], F32)
            nc.vector.memset(eps_t, LN_EPS)

            # initial activation
            x0 = act.tile([128, 4, TLOC], F32R, tag="resid")
            nc.sync.dma_start(out=x0, in_=xt_in.rearrange("(c p) f -> p c f", p=128))

            # DRAM bounce buffers for the per-batch A2A
            sendb = [dram.tile([NCORES, SHARD], BF16, tag=f"send{b}",
                               name=f"send{b}") for b in range(2)]
            recvb = [dram.tile([NCORES, SHARD], BF16, tag=f"recv{b}",
                               name=f"recv{b}") for b in range(2)]

            stat_d = dram.tile([2, DIM], F32, tag="stat")  # LN broadcast bounce

            def project_T(x, w, btile, tag, odt=F32R):
                """(128,4,TLOC) <- relu(w^T x + b), transposed output."""
                out = qkv.tile([128, 4, TLOC], odt, tag=tag)
                for pair in range(2):
                    ps = s_psum.tile([128, 2, TLOC], F32, tag="s")
                    for i in range(2):
                        ofc = 2 * pair + i
                        for ifc in range(4):
                            nc.tensor.matmul(
                                ps[:, i, :],
                                w[:, ifc, 128 * ofc:128 * (ofc + 1)],
                                x[:, ifc, :],
                                start=(ifc == 0), stop=(ifc == 3))
                        nc.vector.tensor_scalar(
                            out=out[:, ofc, :], in0=ps[:, i, :],
                            scalar1=btile[:, ofc:ofc + 1], scalar2=0.0,
                            op0=OP.add, op1=OP.max)
                return out

            def project_V(x):
                """(128,4,DIM) f32r <- relu(x^T wv + bv), natural layout."""
                out = qkv.tile([128, 4, DIM], BF16, tag="v")
                for pair in range(2):
                    ps = s_psum.tile([128, 2, DIM], F32, tag="s")
                    for i in range(2):
                        tch = 2 * pair + i
                        for ifc in range(4):
                            nc.tensor.matmul(
                                ps[:, i, :],
                                x[:, ifc, 128 * tch:128 * (tch + 1)],
                                wv[:, ifc, :],
                                start=(ifc == 0), stop=False)
                        nc.tensor.matmul(
                            ps[:, i, :], ones[0:1, :], bv,
                            start=False, stop=True)
                        nc.vector.tensor_scalar(
                            out=out[:, tch, :], in0=ps[:, i, :],
                            scalar1=0.0, scalar2=None, op0=OP.max)
                return out

            def mha(x):
                qt = project_T(x, wq, bq, "qt", odt=BF16)
                kt = project_T(x, wk, bk, "kt", odt=BF16)
                v = project_V(x)

                # pack + exchange, per batch
                qth, kth, v1 = [], [], []
                for b in range(2):
                    sb, rb = sendb[b], recvb[b]
                    for d in range(NCORES):
                        cq, p0 = d // 2, 64 * (d % 2)
                        nc.sync.dma_start(
                            out=sb[d, 0:16384].rearrange("(r c) -> r c", c=256),
                            in_=qt[p0:p0 + 64, cq, 256 * b:256 * (b + 1)])
                        nc.sync.dma_start(
                            out=sb[d, 16384:32768].rearrange("(r c) -> r c", c=256),
                            in_=kt[p0:p0 + 64, cq, 256 * b:256 * (b + 1)])
                        nc.sync.dma_start(
                            out=sb[d, 32768:49152].rearrange(
                                "(tc p j) -> p tc j", tc=2, p=128),
                            in_=v[:, 2 * b:2 * b + 2, 64 * d:64 * (d + 1)])
                    nc.gpsimd.collective_compute(
                        "AllToAll", OP.bypass, replica_groups=groups,
                        ins=[sb.opt()], outs=[rb.opt()])
                    qh = gath.tile([64, NCORES, 256], BF16, tag=f"qh{b}")
                    kh = gath.tile([64, NCORES, 256], BF16, tag=f"kh{b}")
                    vh = gath.tile([128, 16, 65], BF16, tag=f"vh{b}")
                    nc.sync.dma_start(
                        out=qh, in_=rb[:, 0:16384].rearrange("s (r c) -> r s c", r=64))
                    nc.sync.dma_start(
                        out=kh, in_=rb[:, 16384:32768].rearrange("s (r c) -> r s c", r=64))
                    for tc in range(2):
                        nc.sync.dma_start(
                            out=vh[:, tc::2, 0:64],
                            in_=rb[:, 32768 + 8192 * tc:32768 + 8192 * (tc + 1)]
                                .rearrange("s (p j) -> p s j", p=128))
                    nc.sync.dma_start(out=vh[:, :, 64:65], in_=ones3)
                    qth.append(qh); kth.append(kh); v1.append(vh)

                # attention; Z^T built in-place
                z = qkv.tile([128, 4, TLOC], F32R, tag="z")
                for b in range(2):
                    for j in range(4):
                        ops = o_psum.tile([65, 512], F32, tag="o")
                        for g in range(8):
                            sps = s_psum.tile([128, 2, 512], F32, tag="s")
                            for u in range(2):
                                k = 2 * g + u
                                nc.tensor.matmul(
                                    sps[:, u, :],
                                    kth[b][:, k // 2, 128 * (k % 2):128 * (k % 2 + 1)],
                                    qth[b][:, 2 * j:2 * j + 2, :],
                                    start=True, stop=True)
                            e = epool.tile([128, 2, 512], BF16, tag="e")
                            nc.scalar.activation(e, sps, AF.Exp, scale=0.125)
                            for u in range(2):
                                k = 2 * g + u
                                nc.tensor.matmul(
                                    ops, v1[b][:, k, :], e[:, u, :],
                                    start=(k == 0), stop=(k == 15))
                        rsum = small.tile([1, 512], F32, tag="rsum", bufs=2)
                        nc.vector.tensor_copy(rsum, ops[64:65, :])
                        recip = small.tile([1, 512], F32, tag="recip", bufs=2)
                        nc.vector.reciprocal_approx_fast(recip, rsum)
                        # broadcast recip over 64 partitions via DRAM bounce
                        nc.sync.dma_start(out=stat_d[0:1, :], in_=recip)
                        rrep = small.tile([64, 512], F32, tag="rrep", bufs=2)
                        nc.sync.dma_start(
                            out=rrep,
                            in_=stat_d[0:1, :].partition_broadcast(64)[:, 0, :])
                        o_v = ops[0:64, :].rearrange("f (r s) -> f s r", s=8)
                        r_v = rrep.rearrange("f (r s) -> f s r", s=8)
                        for q in range(2):
                            nc.vector.tensor_tensor(
                                out=z[64 * q:64 * (q + 1), :,
                                      256 * b + 64 * j:256 * b + 64 * (j + 1)],
                                in0=o_v[:, q::2, :],
                                in1=r_v[:, q::2, :],
                                op=OP.mult)
                return project_T(z, wo, bo, "y")

            def layer_norm(x, resid=None):
                """LN over features (partitions); optional residual add first."""
                if resid is not None:
                    xr = act.tile([128, 4, TLOC], F32R, tag="xr")
                    nc.vector.tensor_tensor(out=xr, in0=x, in1=resid, op=OP.add)
                    x = xr
                x2 = act.tile([128, 4, TLOC], F32R, tag="x2")
                nc.vector.tensor_tensor(out=x2, in0=x, in1=x, op=OP.mult)
                mu_ps = ln_psum.tile([1, TLOC], F32, tag="ln")
                for ifc in range(4):
                    nc.tensor.matmul(mu_ps, ones[:, 0:1], x[:, ifc, :],
                                     start=(ifc == 0), stop=(ifc == 3))
                m2_ps = ln_psum.tile([1, TLOC], F32, tag="ln")
                for ifc in range(4):
                    nc.tensor.matmul(m2_ps, ones[:, 0:1], x2[:, ifc, :],
                                     start=(ifc == 0), stop=(ifc == 3))
                mu = small.tile([1, TLOC], F32, tag="mu", bufs=2)
                ex2 = small.tile([1, TLOC], F32, tag="ex2", bufs=2)
                nc.vector.tensor_scalar(out=mu, in0=mu_ps, scalar1=1.0 / DIM,
                                        scalar2=None, op0=OP.mult)
                nc.vector.tensor_scalar(out=ex2, in0=m2_ps, scalar1=1.0 / DIM,
                                        scalar2=None, op0=OP.mult)
                m2 = small.tile([1, TLOC], F32, tag="m2", bufs=2)
                nc.vector.tensor_tensor(out=m2, in0=mu, in1=mu, op=OP.mult)
                nc.vector.tensor_tensor(out=ex2, in0=ex2, in1=m2, op=OP.subtract)
                # rstd = exp(-0.5*ln(var+eps)): Log+Exp share one ACT table set
                # with attention's exp -> no table reloads (Sqrt would thrash).
                sd = small.tile([1, TLOC], F32, tag="sd", bufs=2)
                nc.scalar.activation(sd, ex2, AF.Ln, bias=eps_t)
                nc.scalar.activation(sd, sd, AF.Exp, scale=-0.5)
                nc.vector.tensor_tensor(out=mu, in0=mu, in1=sd, op=OP.mult)
                # broadcast rstd & mu*rstd across partitions via DRAM bounce
                nc.sync.dma_start(out=stat_d[0:1, :], in_=sd)
                nc.sync.dma_start(out=stat_d[1:2, :], in_=mu)
                srep = small.tile([128, 2, TLOC], F32, tag="srep", bufs=2)
                nc.sync.dma_start(
                    out=srep, in_=stat_d.partition_broadcast(128))
                out = act.tile([128, 4, TLOC], F32R, tag="lnout", bufs=2)
                for ifc in range(4):
                    t1 = small.tile([128, TLOC], F32, tag="t1", bufs=2)
                    nc.vector.tensor_tensor(out=t1, in0=x[:, ifc, :],
                                            in1=srep[:, 0, :], op=OP.mult)
                    nc.vector.tensor_tensor(out=t1, in0=t1, in1=srep[:, 1, :],
                                            op=OP.subtract)
                    nc.vector.tensor_scalar(
                        out=out[:, ifc, :], in0=t1,
                        scalar1=lng[:, ifc:ifc + 1], scalar2=lnb[:, ifc:ifc + 1],
                        op0=OP.mult, op1=OP.add)
                return out

            init = x0
            for _ in range(NITER):
                o1 = mha(init)
                l1 = layer_norm(o1)
                o2 = mha(l1)
                init = layer_norm(o2, resid=init)

            nc.sync.dma_start(
                out=out_d.rearrange("(c p) f -> p c f", p=128), in_=init)
    return nc


_NC_CACHE = None


def _get_nc():
    global _NC_CACHE
    if _NC_CACHE is None:
        nc = bacc.Bacc("TRN2", target_bir_lowering=False, debug=False,
                       num_devices=NCORES)
        _build_graph(nc)
        nc.compile()
        _NC_CACHE = nc
    return _NC_CACHE


def kernel(encoder_inputs, Wq, bq, Wk, bk, Wv, bv, Wo, bo, ln_g, ln_b,
           _trace=False, _trace_kwargs=None):
    x = np.asarray(encoder_inputs, dtype=np.float32)
    consts = {
        "wq": np.ascontiguousarray(np.asarray(Wq, np.float32)),
        "wk": np.ascontiguousarray(np.asarray(Wk, np.float32)),
        "wv": np.ascontiguousarray(np.asarray(Wv, np.float32)),
        "wo": np.ascontiguousarray(np.asarray(Wo, np.float32)),
        "bq": np.ascontiguousarray(np.asarray(bq, np.float32).reshape(4, 128).T),
        "bk": np.ascontiguousarray(np.asarray(bk, np.float32).reshape(4, 128).T),
        "bo": np.ascontiguousarray(np.asarray(bo, np.float32).reshape(4, 128).T),
        "bv": np.asarray(bv, np.float32).reshape(1, DIM),
        "lng": np.ascontiguousarray(np.asarray(ln_g, np.float32).reshape(4, 128).T),
        "lnb": np.ascontiguousarray(np.asarray(ln_b, np.float32).reshape(4, 128).T),
        "ones": np.ones((128, 128), np.float32),
        "ones3": np.ones((128, 16, 1), ml_dtypes.bfloat16),
    }
    in_maps = []
    for c in range(NCORES):
        xt = np.concatenate([x[0, 256 * c:256 * (c + 1)].T,
                             x[1, 256 * c:256 * (c + 1)].T], axis=1)
        in_maps.append({"xt": np.ascontiguousarray(xt), **consts})

    nc = _get_nc()
    res = bass_utils.run_bass_kernel_spmd(
        nc, in_maps, core_ids=list(range(NCORES)),
        trace=_trace, **(_trace_kwargs or {}))

    out = np.zeros((2, 2048, DIM), np.float32)
    for c in range(NCORES):
        r = res.results[c]["out"]
        out[0, 256 * c:256 * (c + 1)] = r[:, :256].T
        out[1, 256 * c:256 * (c + 1)] = r[:, 256:].T
    if _trace:
        kernel._last_results = res
    return out
